# revision 1
# baseline (speedup 1.0000x reference)
"""MoE top-2 routing kernel for Trainium2 (8 NeuronCores, expert-parallel).

Strategy: the gating network (softmax over 8 experts + top-2 + renorm) is tiny
(8192x8) and runs on host. Token dispatch happens on host: for each expert e,
gather the tokens that route to e (zero-padded to a fixed capacity), and core e
computes one dense GEMM  Ye = Xg_e @ W_e^T  in float32r (full-rate PE fp32
mode). Host combines:  Y[t] += w_te * (Ye[slot] + b_e).

Only 2/8 of the dense all-expert compute is performed (top-2 routing), split
one-expert-per-core. All device-side tensors are host-pre-tiled so every DMA
descriptor is a large contiguous block per SBUF partition.
"""

import os

import numpy as np

N_TOK, N_EXP, D_IN, D_OUT = 8192, 8, 2048, 2048
TOP_K = 2

P = 128
MT = 256                 # token tile (M dim of the GEMM)
MSUB = MT // P           # PSUM partition groups per token tile
NT = 512                 # out-feature tile (PSUM free dim)

LAST_EXEC_NS = None  # set when KERNEL_TRACE=1

_cache = {}


def _install_ntff_shim():
    """Provide antenv.axon_hooks (missing in this image) so trace=True works."""
    import sys
    import types

    if "antenv.axon_hooks" in sys.modules:
        return
    try:
        import antenv
        from trn_agent_boot.trn_boot import _ntff_profile_via_ctypes

        mod = types.ModuleType("antenv.axon_hooks")
        mod._hook = _ntff_profile_via_ctypes("/opt/axon/libaxon_pjrt.so")
        mod.set_axon_ntff_profile_hook = lambda h: setattr(mod, "_hook", h)
        mod.get_axon_ntff_profile_hook = lambda: mod._hook
        sys.modules["antenv.axon_hooks"] = mod
        antenv.axon_hooks = mod
    except Exception:
        pass


def _build_v2(cap, d_in, d_out):
    """Custom tile kernel: out = Xg @ We^T, W SBUF-resident, f32r matmuls.

    DRAM layouts (host pre-tiled, contiguous per SBUF partition):
      xg : [m_tiles, P, KSUB, MT]   xg[m,p,ks,j] = Xg[m*MT+j, ks*P+p]
      w  : [n_tiles, P, KSUB, NT]   w[n,p,ks,c]  = W_e[n*NT+c, ks*P+p]
      out: [m_tiles, P, n_tiles, MSUB, NT]  out[m,p,n,ms,c] = Ye[m*MT+ms*P+p, n*NT+c]
    """
    import concourse.mybir as mybir
    import concourse.tile as tile
    from concourse import bacc

    KSUB = d_in // P
    n_tiles = d_out // NT
    m_tiles = cap // MT

    nc = bacc.Bacc("TRN2", target_bir_lowering=False, debug=False)
    with tile.TileContext(nc) as tc:
        with (
            tc.tile_pool(name="dram", bufs=1, space="DRAM") as dram,
            tc.tile_pool(name="wpool", bufs=1) as wpool,
            tc.tile_pool(name="xpool", bufs=3) as xpool,
            tc.tile_pool(name="opool", bufs=3) as opool,
            tc.tile_pool(name="pspool", bufs=2, space="PSUM") as pspool,
            tc.tile_pool(name="warmpool", bufs=1) as warmpool,
            tc.tile_pool(name="warmps", bufs=1, space="PSUM") as warmps,
        ):
            xg = dram.tile([m_tiles, P, KSUB, MT], mybir.dt.float32r,
                           kind="ExternalInput", name="xg")
            w = dram.tile([n_tiles, P, KSUB, NT], mybir.dt.float32r,
                          kind="ExternalInput", name="w")
            out = dram.tile([m_tiles, P, n_tiles, MSUB, NT], mybir.dt.float32,
                            kind="ExternalOutput", name="out")

            # PE warmup: dummy matmuls on zeroed scratch run during the initial
            # DMA wait (PE is otherwise idle ~8-14us) so the HAM clock gate is
            # at 2.4GHz when the first real matmuls issue.
            wl = warmpool.tile([P, P], mybir.dt.float32, name="warm_l")
            wr = warmpool.tile([P, NT], mybir.dt.float32, name="warm_r")
            nc.gpsimd.memset(wl[:], 0.0)
            nc.gpsimd.memset(wr[:], 0.0)
            wp = warmps.tile([P, NT], mybir.dt.float32, name="warm_p")
            for _ in range(8):
                nc.tensor.matmul(wp[:], lhsT=wl[:], rhs=wr[:], start=True, stop=True,
                                 skip_group_check=True)

            # W resident in SBUF. w0 first (first blocks need it), then xg0..xg2,
            # then the rest of W — matches the interleaved first-block order.
            # The very first loads are split per k-chunk so the first matmul
            # chain can start as soon as its k=0 slices land (range-precise deps).
            wtiles = [wpool.tile([P, KSUB, NT], mybir.dt.float32r, name=f"wt{n}")
                      for n in range(n_tiles)]

            xtiles = {}

            def load_x(m):
                xt = xpool.tile([P, KSUB, MT], mybir.dt.float32r, name="xt")
                nc.sync.dma_start(out=xt[:], in_=xg[m])
                xtiles[m] = xt

            # Earliest-deadline-first emission. w0/xg0 interleaved in k-quarters
            # so the first accumulation chain starts as soon as the k=0 slices
            # land (deps are range-precise); later tiles as whole DMAs — more
            # splits delay the W burst (~1.5us sequencer descgen per dma_start).
            q = KSUB // 4
            xt0 = xpool.tile([P, KSUB, MT], mybir.dt.float32r, name="xt")
            xtiles[0] = xt0
            for k0 in range(0, KSUB, q):
                nc.sync.dma_start(out=wtiles[0][:, k0:k0 + q], in_=w[0, :, k0:k0 + q])
                nc.sync.dma_start(out=xt0[:, k0:k0 + q], in_=xg[0, :, k0:k0 + q])
            n_lead = min(3, m_tiles)
            for m in range(1, n_lead):
                load_x(m)
            for n in range(1, n_tiles):
                nc.sync.dma_start(out=wtiles[n][:], in_=w[n])

            # Interleave the first n_lead m-tiles across n so early compute
            # overlaps the DMA of w1..w3; then m-major order.
            blocks = []
            for n in range(n_tiles):
                blocks += [(m, n) for m in range(n_lead)]
            for m in range(n_lead, m_tiles):
                for n in range(n_tiles):
                    blocks.append((m, n))

            done_n = {}  # m -> number of (m, n) blocks emitted
            for bi, (m, n) in enumerate(blocks):
                if m not in xtiles:
                    load_x(m)
                xt = xtiles[m]
                last_block = bi == len(blocks) - 1
                ot = opool.tile([P, MSUB, NT], mybir.dt.float32, name="ot")
                for ms in range(MSUB):
                    ps = pspool.tile([P, NT], mybir.dt.float32, name=f"ps{ms}")
                    for k in range(KSUB):
                        nc.tensor.matmul(
                            ps[:],
                            lhsT=xt[:, k, ms * P:(ms + 1) * P],
                            rhs=wtiles[n][:, k, :],
                            start=(k == 0),
                            stop=(k == KSUB - 1),
                        )
                    nc.vector.tensor_copy(ot[:, ms, :], ps[:])
                    if last_block:
                        # split the final store per-ms so it overlaps the
                        # last eviction instead of trailing the whole block
                        nc.sync.dma_start(out=out[m, :, n, ms], in_=ot[:, ms])
                if not last_block:
                    nc.sync.dma_start(out=out[m, :, n], in_=ot[:])
                done_n[m] = done_n.get(m, 0) + 1
                if done_n[m] == n_tiles:
                    del xtiles[m]  # release the slot for the next prefetch

    nc.compile()
    return nc, xg.name, w.name, out.name


def kernel(X, G, W, b):
    global LAST_EXEC_NS
    from concourse.bass_utils import run_bass_kernel_spmd

    X = np.ascontiguousarray(np.asarray(X, dtype=np.float32))
    G = np.asarray(G, dtype=np.float32)
    W = np.asarray(W, dtype=np.float32)
    b = np.asarray(b, dtype=np.float32)
    n_tok, d_in = X.shape
    n_exp = G.shape[1]
    d_out = W.shape[1]
    ksub = d_in // P
    n_tiles = d_out // NT

    # --- host gating: softmax over experts, top-2, renormalize ---
    g = G - G.max(axis=1, keepdims=True)
    sm = np.exp(g)
    sm /= sm.sum(axis=1, keepdims=True)
    top_idx = np.argsort(-sm, axis=1, kind="stable")[:, :TOP_K]  # ties -> lower index
    top_w = np.take_along_axis(sm, top_idx, axis=1)
    norm_w = top_w / top_w.sum(axis=1, keepdims=True)

    # --- token dispatch (host): per expert, gather routed tokens ---
    exp_tokens = []
    exp_scales = []
    for e in range(n_exp):
        masks = [top_idx[:, k] == e for k in range(TOP_K)]
        idx = np.concatenate([np.where(m)[0] for m in masks])
        s = np.concatenate([norm_w[m, k] for k, m in enumerate(masks)])
        exp_tokens.append(idx)
        exp_scales.append(s.astype(np.float32))

    max_cnt = max(len(i) for i in exp_tokens)
    cap = max(MT, -(-max_cnt // MT) * MT)
    m_tiles = cap // MT

    _install_ntff_shim()  # harmless if unavailable; needed if tracing is on
    key = (cap, d_in, d_out)
    if key not in _cache:
        _cache[key] = _build_v2(cap, d_in, d_out)
    nc, xg_name, w_name, out_name = _cache[key]

    in_maps = []
    for e in range(n_exp):
        idx = exp_tokens[e]
        Xg = np.zeros((cap, d_in), dtype=np.float32)
        Xg[: len(idx)] = X[idx]
        # [cap, d_in] -> [m_tiles, P, KSUB, MT]
        xg_t = np.ascontiguousarray(
            Xg.reshape(m_tiles, MT, ksub, P).transpose(0, 3, 2, 1))
        # W_e [d_out, d_in] -> [n_tiles, P, KSUB, NT]
        w_t = np.ascontiguousarray(
            W[e].reshape(n_tiles, NT, ksub, P).transpose(0, 3, 2, 1))
        in_maps.append({xg_name: xg_t, w_name: w_t})

    trace = bool(os.environ.get("KERNEL_TRACE"))
    res = run_bass_kernel_spmd(nc, in_maps, core_ids=list(range(n_exp)), trace=trace)
    LAST_EXEC_NS = res.exec_time_ns

    # --- host combine: scatter-add with gate scale and bias ---
    Y = np.zeros((n_tok, d_out), dtype=np.float32)
    for e in range(n_exp):
        idx = exp_tokens[e]
        if len(idx) == 0:
            continue
        arr = res.results[e][out_name]  # [m_tiles, P, n_tiles, MSUB, NT]
        Ye = arr.transpose(0, 3, 1, 2, 4).reshape(cap, d_out)[: len(idx)]
        s = exp_scales[e][:, None]
        Y[idx] += s * (Ye + b[e][None, :])
    return Y



# revision 6
# speedup vs baseline: 1.4936x; 1.4936x over previous
"""MoE top-2 routing kernel for Trainium2 (8 NeuronCores, expert-parallel).

Strategy: the gating network (softmax over 8 experts + top-2 + renorm) is tiny
(8192x8) and runs on host. Token dispatch happens on host: for each expert e,
gather the tokens that route to e (zero-padded to a fixed capacity), and core e
computes one dense GEMM  Ye = Xg_e @ W_e^T  in float32r (full-rate PE fp32
mode). Host combines:  Y[t] += w_te * (Ye[slot] + b_e).

Only 2/8 of the dense all-expert compute is performed (top-2 routing), split
one-expert-per-core. All device-side tensors are host-pre-tiled so every DMA
descriptor is a large contiguous block per SBUF partition.
"""

import os

import numpy as np

N_TOK, N_EXP, D_IN, D_OUT = 8192, 8, 2048, 2048
TOP_K = 2

P = 128
MT = 256                 # token tile (M dim of the GEMM)
MSUB = MT // P           # PSUM partition groups per token tile
NT = 512                 # out-feature tile (PSUM free dim)

LAST_EXEC_NS = None  # set when KERNEL_TRACE=1

_cache = {}


def _install_ntff_shim():
    """Provide antenv.axon_hooks (missing in this image) so trace=True works."""
    import sys
    import types

    if "antenv.axon_hooks" in sys.modules:
        return
    try:
        import antenv
        from trn_agent_boot.trn_boot import _ntff_profile_via_ctypes

        mod = types.ModuleType("antenv.axon_hooks")
        mod._hook = _ntff_profile_via_ctypes("/opt/axon/libaxon_pjrt.so")
        mod.set_axon_ntff_profile_hook = lambda h: setattr(mod, "_hook", h)
        mod.get_axon_ntff_profile_hook = lambda: mod._hook
        sys.modules["antenv.axon_hooks"] = mod
        antenv.axon_hooks = mod
    except Exception:
        pass


def _build_v2(cap, d_in, d_out):
    """Custom tile kernel: out = Xg @ We^T, W SBUF-resident, f32r matmuls.

    DRAM layouts (host pre-tiled, contiguous per SBUF partition):
      xg : [m_tiles, P, KSUB, MT]   xg[m,p,ks,j] = Xg[m*MT+j, ks*P+p]
      w  : [n_tiles, P, KSUB, NT]   w[n,p,ks,c]  = W_e[n*NT+c, ks*P+p]
      out: [m_tiles, P, n_tiles, MSUB, NT]  out[m,p,n,ms,c] = Ye[m*MT+ms*P+p, n*NT+c]
    """
    import concourse.mybir as mybir
    import concourse.tile as tile
    from concourse import bacc

    KSUB = d_in // P
    n_tiles = d_out // NT
    m_tiles = cap // MT

    nc = bacc.Bacc("TRN2", target_bir_lowering=False, debug=False)
    with tile.TileContext(nc) as tc:
        with (
            tc.tile_pool(name="dram", bufs=1, space="DRAM") as dram,
            tc.tile_pool(name="wpool", bufs=1) as wpool,
            tc.tile_pool(name="xpool", bufs=3) as xpool,
            tc.tile_pool(name="opool", bufs=3) as opool,
            tc.tile_pool(name="pspool", bufs=2, space="PSUM") as pspool,
            tc.tile_pool(name="warmpool", bufs=1) as warmpool,
            tc.tile_pool(name="warmps", bufs=1, space="PSUM") as warmps,
        ):
            xg = dram.tile([m_tiles, P, KSUB, MT], mybir.dt.bfloat16,
                           kind="ExternalInput", name="xg")
            w = dram.tile([n_tiles, P, KSUB, NT], mybir.dt.bfloat16,
                          kind="ExternalInput", name="w")
            out = dram.tile([m_tiles, P, n_tiles, MSUB, NT], mybir.dt.float32,
                            kind="ExternalOutput", name="out")

            # PE warmup: dummy matmuls on zeroed scratch run during the initial
            # DMA wait (PE is otherwise idle ~8-14us) so the HAM clock gate is
            # at 2.4GHz when the first real matmuls issue.
            wl = warmpool.tile([P, P], mybir.dt.float32, name="warm_l")
            wr = warmpool.tile([P, NT], mybir.dt.float32, name="warm_r")
            nc.gpsimd.memset(wl[:], 0.0)
            nc.gpsimd.memset(wr[:], 0.0)
            wp = warmps.tile([P, NT], mybir.dt.float32, name="warm_p")
            for _ in range(8):
                nc.tensor.matmul(wp[:], lhsT=wl[:], rhs=wr[:], start=True, stop=True,
                                 skip_group_check=True)

            # W resident in SBUF. w0 first (first blocks need it), then xg0..xg2,
            # then the rest of W — matches the interleaved first-block order.
            # The very first loads are split per k-chunk so the first matmul
            # chain can start as soon as its k=0 slices land (range-precise deps).
            wtiles = [wpool.tile([P, KSUB, NT], mybir.dt.bfloat16, name=f"wt{n}")
                      for n in range(n_tiles)]

            xtiles = {}

            def load_x(m):
                xt = xpool.tile([P, KSUB, MT], mybir.dt.bfloat16, name="xt")
                nc.sync.dma_start(out=xt[:], in_=xg[m])
                xtiles[m] = xt

            # Earliest-deadline-first emission. w0/xg0 interleaved in k-quarters
            # so the first accumulation chain starts as soon as the k=0 slices
            # land (deps are range-precise); later tiles as whole DMAs — more
            # splits delay the W burst (~1.5us sequencer descgen per dma_start).
            q = KSUB // 4
            xt0 = xpool.tile([P, KSUB, MT], mybir.dt.bfloat16, name="xt")
            xtiles[0] = xt0
            for k0 in range(0, KSUB, q):
                nc.sync.dma_start(out=wtiles[0][:, k0:k0 + q], in_=w[0, :, k0:k0 + q])
                nc.sync.dma_start(out=xt0[:, k0:k0 + q], in_=xg[0, :, k0:k0 + q])
            n_lead = min(3, m_tiles)
            for m in range(1, n_lead):
                load_x(m)
            for n in range(1, n_tiles):
                nc.sync.dma_start(out=wtiles[n][:], in_=w[n])

            # Interleave the first n_lead m-tiles across n so early compute
            # overlaps the DMA of w1..w3; then m-major order.
            blocks = []
            for n in range(n_tiles):
                blocks += [(m, n) for m in range(n_lead)]
            for m in range(n_lead, m_tiles):
                for n in range(n_tiles):
                    blocks.append((m, n))

            done_n = {}  # m -> number of (m, n) blocks emitted
            for bi, (m, n) in enumerate(blocks):
                if m not in xtiles:
                    load_x(m)
                xt = xtiles[m]
                last_block = bi == len(blocks) - 1
                ot = opool.tile([P, MSUB, NT], mybir.dt.float32, name="ot")
                for ms in range(MSUB):
                    ps = pspool.tile([P, NT], mybir.dt.float32, name=f"ps{ms}")
                    for k in range(KSUB):
                        nc.tensor.matmul(
                            ps[:],
                            lhsT=xt[:, k, ms * P:(ms + 1) * P],
                            rhs=wtiles[n][:, k, :],
                            start=(k == 0),
                            stop=(k == KSUB - 1),
                        )
                    nc.vector.tensor_copy(ot[:, ms, :], ps[:])
                    if last_block:
                        # split the final store per-ms so it overlaps the
                        # last eviction instead of trailing the whole block
                        nc.sync.dma_start(out=out[m, :, n, ms], in_=ot[:, ms])
                if not last_block:
                    nc.sync.dma_start(out=out[m, :, n], in_=ot[:])
                done_n[m] = done_n.get(m, 0) + 1
                if done_n[m] == n_tiles:
                    del xtiles[m]  # release the slot for the next prefetch

    nc.compile()
    return nc, xg.name, w.name, out.name


def kernel(X, G, W, b):
    global LAST_EXEC_NS
    from concourse.bass_utils import run_bass_kernel_spmd

    X = np.ascontiguousarray(np.asarray(X, dtype=np.float32))
    G = np.asarray(G, dtype=np.float32)
    W = np.asarray(W, dtype=np.float32)
    b = np.asarray(b, dtype=np.float32)
    n_tok, d_in = X.shape
    n_exp = G.shape[1]
    d_out = W.shape[1]
    ksub = d_in // P
    n_tiles = d_out // NT

    # --- host gating: softmax over experts, top-2, renormalize ---
    g = G - G.max(axis=1, keepdims=True)
    sm = np.exp(g)
    sm /= sm.sum(axis=1, keepdims=True)
    top_idx = np.argsort(-sm, axis=1, kind="stable")[:, :TOP_K]  # ties -> lower index
    top_w = np.take_along_axis(sm, top_idx, axis=1)
    norm_w = top_w / top_w.sum(axis=1, keepdims=True)

    # --- token dispatch (host): per expert, gather routed tokens ---
    exp_tokens = []
    exp_scales = []
    for e in range(n_exp):
        masks = [top_idx[:, k] == e for k in range(TOP_K)]
        idx = np.concatenate([np.where(m)[0] for m in masks])
        s = np.concatenate([norm_w[m, k] for k, m in enumerate(masks)])
        exp_tokens.append(idx)
        exp_scales.append(s.astype(np.float32))

    max_cnt = max(len(i) for i in exp_tokens)
    cap = max(MT, -(-max_cnt // MT) * MT)
    m_tiles = cap // MT

    _install_ntff_shim()  # harmless if unavailable; needed if tracing is on
    key = (cap, d_in, d_out)
    if key not in _cache:
        _cache[key] = _build_v2(cap, d_in, d_out)
    nc, xg_name, w_name, out_name = _cache[key]

    import ml_dtypes

    in_maps = []
    for e in range(n_exp):
        idx = exp_tokens[e]
        Xg = np.zeros((cap, d_in), dtype=np.float32)
        Xg[: len(idx)] = X[idx]
        # [cap, d_in] -> [m_tiles, P, KSUB, MT]
        xg_t = np.ascontiguousarray(
            Xg.reshape(m_tiles, MT, ksub, P).transpose(0, 3, 2, 1)
        ).astype(ml_dtypes.bfloat16)
        # W_e [d_out, d_in] -> [n_tiles, P, KSUB, NT]
        w_t = np.ascontiguousarray(
            W[e].reshape(n_tiles, NT, ksub, P).transpose(0, 3, 2, 1)
        ).astype(ml_dtypes.bfloat16)
        in_maps.append({xg_name: xg_t, w_name: w_t})

    trace = bool(os.environ.get("KERNEL_TRACE"))
    res = run_bass_kernel_spmd(nc, in_maps, core_ids=list(range(n_exp)), trace=trace)
    LAST_EXEC_NS = res.exec_time_ns

    # --- host combine: scatter-add with gate scale and bias ---
    Y = np.zeros((n_tok, d_out), dtype=np.float32)
    for e in range(n_exp):
        idx = exp_tokens[e]
        if len(idx) == 0:
            continue
        arr = res.results[e][out_name]  # [m_tiles, P, n_tiles, MSUB, NT]
        Ye = arr.transpose(0, 3, 1, 2, 4).reshape(cap, d_out)[: len(idx)]
        s = exp_scales[e][:, None]
        Y[idx] += s * (Ye + b[e][None, :])
    return Y

